# revision 12
# baseline (speedup 1.0000x reference)
"""Fused causal MHA (RoPE) Trainium2 Bass kernel, SPMD over 8 NeuronCores.

Sharding: data-parallel over batch (4) x tensor-parallel over heads (2 groups
of 8 heads).  Core c handles batch c//2, heads (c%2)*8 .. +8.  Each core
computes a partial output (its 8 heads through the row-sharded Wo); the host
sums the two partials per batch.

v3 = fast fp8 DoubleRow pipeline for queries 512..2048 plus an fp16 "patch"
phase that recomputes queries 0..512 at high precision (attention there is
peaked on few keys, so fp8 quantization noise shows up directly in the
output; beyond 512 keys the softmax averaging washes it out).

fp8 main pipeline (all matmuls DoubleRow fp8e4m3, 0.5 cyc/elem):
  - projections contract d_model in pairs of 128-chunks (true DR pairs)
  - scores use the broadcast trick: both DR k-tiles alias the same k/q via
    stride-0 APs; q tables are pre-scaled by 0.5 so the x2 cancels
  - causal mask added with a 0.5*I stationary (broadcast DR), mask = -240
  - PV contracts two key tiles per instruction; row 64 of the stationary
    holds ones -> softmax denominators for free
  - output projection contracts E in DR pairs on fp8 o
RoPE: PE emits the projection and a row-swapped projection (row-permuted
fp8 weight copies); DVE needs only 2 muls + 1 add per chunk.  Head rows are
stored 16-interleaved ([x1a x2a x1b x2b] per head) so the patch phase can
swap partners with DVE stream_shuffle instead of a second projection.
exp on ACT (PSUM->SBUF, scale 1/8, bias -1.5); the q<256 quarter of each
diagonal 512-block skips exp (Pool memsets it).  Normalization: DVE copy +
reciprocal, Pool broadcast + multiply.  Phases interleave to keep PE fed.
"""

import numpy as np
import ml_dtypes

B = 4
S = 2048
D = 1024
H = 16
DK = 64
THETA = 10000.0
N_CORES = 8
E = 512          # per-core head-dim shard (8 heads * 64)
NHP = 4          # head pairs per core
ST = S // 128    # seq tiles of 128
DC = D // 128    # d_model chunks of 128
SP = 512         # patch length (queries 0..SP recomputed in fp16)
MASK_NEG = -240.0
F8 = ml_dtypes.float8_e4m3

_cache = {}


def _build_program():
    import concourse.tile as tile
    from concourse import bacc, mybir
    from contextlib import ExitStack

    f8 = mybir.dt.float8e4
    f16 = mybir.dt.float16
    f32 = mybir.dt.float32
    DR = mybir.MatmulPerfMode.DoubleRow
    EXP = mybir.ActivationFunctionType.Exp

    nc = bacc.Bacc("TRN2", target_bir_lowering=False, debug=False,
                   num_devices=N_CORES)

    def din(name, shape, dt=f8):
        return nc.dram_tensor(name, shape, dt, kind="ExternalInput").ap()

    xt = din("xt", [128, DC, S])               # x[b]^T as [128, dchunk, s]
    wq = din("wq", [128, DC, E])               # Wq' (permuted) ^T
    wqs = din("wqs", [128, DC, E])             # partner-swapped rows
    wk = din("wk", [128, DC, E])
    wks = din("wks", [128, DC, E])
    wv = din("wv", [128, DC, E])
    wo = din("wo", [128, E // 128, D])         # Wo'^T chunks [128e, 1024d]
    ctk = din("ctk", [128, S], f16)            # cos table
    stk = din("stk", [128, S], f16)            # signed sin table
    masks = din("masks", [128, 4, 512])        # additive causal masks (-240/0)
    ident = din("ident", [128, 128])           # 0.5*I
    # fp16 patch inputs
    xt16 = din("xt16", [128, DC, SP], f16)
    wq16 = din("wq16", [128, DC, E], f16)
    wk16 = din("wk16", [128, DC, E], f16)
    wv16 = din("wv16", [128, DC, E], f16)
    wo16 = din("wo16", [128, E // 128, D], f16)
    yt = nc.dram_tensor("yt", [128, DC, S - SP], f16, kind="ExternalOutput").ap()
    yt2 = nc.dram_tensor("yt2", [128, DC, SP], f16, kind="ExternalOutput").ap()

    with tile.TileContext(nc) as tc, ExitStack() as ctx:
        sb = ctx.enter_context(tc.tile_pool(name="sb", bufs=1))
        ex_pool = ctx.enter_context(tc.tile_pool(name="exps", bufs=6))
        ex16_pool = ctx.enter_context(tc.tile_pool(name="exps16", bufs=3))
        rope_tmp = ctx.enter_context(tc.tile_pool(name="rtmp", bufs=4))
        patch_tmp = ctx.enter_context(tc.tile_pool(name="ptmp", bufs=2))
        shf_pool = ctx.enter_context(tc.tile_pool(name="shf", bufs=2))
        oraw_pool = ctx.enter_context(tc.tile_pool(name="oraw", bufs=2))
        r_pool = ctx.enter_context(tc.tile_pool(name="rp", bufs=2))
        rb_pool = ctx.enter_context(tc.tile_pool(name="rbp", bufs=2))
        yst_pool = ctx.enter_context(tc.tile_pool(name="yst", bufs=3))

        # ---- resident SBUF tensors ----
        s_xt = sb.tile([128, DC, S], f8)
        s_wq = sb.tile([128, DC, E], f8)
        s_wqs = sb.tile([128, DC, E], f8)
        s_wk = sb.tile([128, DC, E], f8)
        s_wks = sb.tile([128, DC, E], f8)
        s_wv = sb.tile([128, DC, E], f8)
        s_wo = sb.tile([128, E // 128, D], f8)
        s_ck = sb.tile([128, S], f16)
        s_sk = sb.tile([128, S], f16)
        s_m = sb.tile([128, 4, 512], f8)
        s_id = sb.tile([128, 128], f8)
        v_aug = sb.tile([128, ST, 8, 80], f8)    # [k-part, ktile, head, dv+1]
        q_all = sb.tile([128, NHP, S], f8)
        k_all = sb.tile([128, NHP, S], f8)
        o_all = sb.tile([128, NHP, S], f8)       # normalized attn out
        sbias = sb.tile([128, 1], f32)
        # patch tensors
        s_xt16 = sb.tile([128, DC, SP], f16)
        s_wq16 = sb.tile([128, DC, E], f16)
        s_wk16 = sb.tile([128, DC, E], f16)
        s_wv16 = sb.tile([128, DC, E], f16)
        s_wo16 = sb.tile([128, E // 128, D], f16)
        v16 = sb.tile([128, SP // 128, 8, 72], f16)
        q16 = sb.tile([128, NHP, SP], f16)
        k16 = sb.tile([128, NHP, SP], f16)
        o16 = sb.tile([128, NHP, SP], f16)

        for d in range(DC):
            eng = nc.sync if d % 2 == 0 else nc.scalar
            eng.dma_start(s_xt[:, d, :], xt[:, d, :])
        nc.scalar.dma_start(s_wv[:], wv[:])
        for i, (dst, src) in enumerate(
                [(s_wq, wq), (s_wqs, wqs), (s_wk, wk), (s_wks, wks),
                 (s_ck, ctk), (s_sk, stk), (s_m, masks), (s_id, ident),
                 (s_xt16, xt16), (s_wq16, wq16), (s_wk16, wk16),
                 (s_wv16, wv16), (s_wo, wo), (s_wo16, wo16)]):
            eng = nc.sync if i % 2 == 0 else nc.scalar
            eng.dma_start(dst[:], src[:])

        nc.vector.memset(sbias[:], -1.5)
        nc.vector.memset(
            v_aug.rearrange("p a h c -> p (a h) c")[:, :, 64:65], 1.0)
        nc.vector.memset(
            v16.rearrange("p a h c -> p (a h) c")[:, :, 64:65], 1.0)

        pqk = ctx.enter_context(tc.tile_pool(name="pqk", bufs=2, space="PSUM"))
        psc = ctx.enter_context(tc.tile_pool(name="psc", bufs=2, space="PSUM"))
        po = ctx.enter_context(tc.tile_pool(name="po", bufs=1, space="PSUM"))

        SWAP16 = [i ^ 16 for i in range(32)]

        def bc2(ap):
            shp = list(ap.shape)
            for i in range(len(shp) - 1, 0, -1):
                if shp[i] == 1:
                    ap = ap.squeeze(i)
                    shp.pop(i)
            return ap.unsqueeze(1).broadcast_to([shp[0], 2] + shp[1:])

        def vproj(st_i):
            # v = x @ Wv^T for one 128-seq tile (natural layout), fp8 DR
            ps = pqk.tile([128, 512], f32, tag="pq", name="pq")
            for dp in range(DC // 2):
                nc.tensor.matmul(
                    ps[:], s_xt[:, 2 * dp:2 * dp + 2, st_i * 128:(st_i + 1) * 128],
                    s_wv[:, 2 * dp:2 * dp + 2, :], perf_mode=DR,
                    start=(dp == 0), stop=(dp == DC // 2 - 1))
            nc.vector.tensor_copy(
                v_aug[:, st_i, :, 0:64],
                ps.rearrange("p (h v) -> p h v", h=8))

        def projchunk(hp, sc, do_q):
            # fp8 q and/or k (+ row-swapped copies) for one 256-chunk + RoPE
            sl = slice(sc * 256, (sc + 1) * 256)
            cols = slice(hp * 128, (hp + 1) * 128)
            plan = ([(s_wq, s_wqs, q_all)] if do_q else []) + \
                   [(s_wk, s_wks, k_all)]
            for w_n, w_s, dst in plan:
                ps = pqk.tile([128, 512], f32, tag="pq", name="pq")
                for dp in range(DC // 2):
                    nc.tensor.matmul(
                        ps[:, 0:256], w_n[:, 2 * dp:2 * dp + 2, cols],
                        s_xt[:, 2 * dp:2 * dp + 2, sl], perf_mode=DR,
                        start=(dp == 0), stop=(dp == DC // 2 - 1))
                for dp in range(DC // 2):
                    nc.tensor.matmul(
                        ps[:, 256:512], w_s[:, 2 * dp:2 * dp + 2, cols],
                        s_xt[:, 2 * dp:2 * dp + 2, sl], perf_mode=DR,
                        start=(dp == 0), stop=(dp == DC // 2 - 1))
                t = rope_tmp.tile([128, 256], f16, tag="rt", name="rt")
                m = rope_tmp.tile([128, 256], f16, tag="rm", name="rm")
                nc.vector.tensor_mul(t[:], ps[:, 0:256], s_ck[:, sl])
                nc.vector.tensor_mul(m[:], ps[:, 256:512], s_sk[:, sl])
                nc.vector.tensor_add(dst[:, hp, sl], t[:], m[:])

        def norm_chain(ps_o, hp, qc_sl, dst):
            o_raw = oraw_pool.tile([65, 1024], f32, tag="or", name="or")
            nc.vector.tensor_copy(o_raw[:], ps_o[:])       # frees the bank
            r = r_pool.tile([1, 1024], f32, tag="r", name="r")
            nc.vector.reciprocal(r[:], o_raw[64:65, :])
            rb = rb_pool.tile([64, 1024], f32, tag="rb", name="rb")
            nc.gpsimd.partition_broadcast(rb[:], r[:])
            for h in range(2):
                nc.gpsimd.tensor_mul(
                    dst[64 * h:64 * (h + 1), hp, qc_sl],
                    o_raw[0:64, h * 512:(h + 1) * 512],
                    rb[:, h * 512:(h + 1) * 512])

        def attention_qc(hp, qc):
            # fp8 attention for queries qc*512..(qc+1)*512, qc in 1..3
            ps_o = po.tile([65, 1024], f32, tag="po", name="po")
            prev = None

            def pv(pktp, pex, stop):
                for h in range(2):
                    nc.tensor.matmul(
                        ps_o[0:65, h * 512:(h + 1) * 512],
                        v_aug[:, 2 * pktp:2 * pktp + 2, 2 * hp + h, 0:65],
                        pex[h][:], perf_mode=DR,
                        start=(pktp == 0), stop=stop)

            for ktp in range(2 * (qc + 1)):
                ps_s = [psc.tile([128, 2, 512], f32, tag="ps", name=f"ps{h}")
                        for h in range(2)]
                exs = [ex_pool.tile([128, 2, 512], f8, tag="ex", name=f"ex{h}")
                       for h in range(2)]
                for h in range(2):
                    kq = slice(64 * h, 64 * (h + 1))
                    for half in range(2):
                        kt = 2 * ktp + half
                        d = kt - 4 * qc
                        nc.tensor.matmul(
                            ps_s[h][:, half, :],
                            bc2(k_all[kq, hp, kt * 128:(kt + 1) * 128]),
                            bc2(q_all[kq, hp, qc * 512:(qc + 1) * 512]),
                            perf_mode=DR, start=True, stop=(d < 0))
                        if d >= 0:
                            nc.tensor.matmul(
                                ps_s[h][:, half, :], bc2(s_id[:]),
                                bc2(s_m[:, d, :]), perf_mode=DR,
                                start=False, stop=True)
                for h in range(2):
                    if ktp == 2 * qc + 1:
                        nc.gpsimd.memset(exs[h][:, :, 0:256], 0.0)
                        nc.scalar.activation(
                            out=exs[h][:, :, 256:512],
                            in_=ps_s[h][:, :, 256:512],
                            func=EXP, scale=0.125, bias=sbias[:])
                    else:
                        nc.scalar.activation(
                            out=exs[h][:], in_=ps_s[h][:],
                            func=EXP, scale=0.125, bias=sbias[:])
                if prev is not None:
                    pv(prev[0], prev[1], False)
                prev = (ktp, exs)
            pv(prev[0], prev[1], True)
            norm_chain(ps_o, hp, slice(qc * 512, (qc + 1) * 512), o_all)

        def phase_e(sc):
            # fp8 DR output projection for seq chunk sc (1..3) -> yt
            sl = slice(sc * 512, (sc + 1) * 512)
            for mt in range(DC):
                ps = pqk.tile([128, 512], f32, tag="pq", name="pq")
                for ep in range(2):
                    nc.tensor.matmul(
                        ps[:], s_wo[:, 2 * ep:2 * ep + 2, mt * 128:(mt + 1) * 128],
                        o_all[:, 2 * ep:2 * ep + 2, sl], perf_mode=DR,
                        start=(ep == 0), stop=(ep == 1))
                yo = yst_pool.tile([128, 512], f16, tag="yo", name="yo")
                nc.vector.tensor_copy(yo[:], ps[:])
                nc.sync.dma_start(yt[:, mt, (sc - 1) * 512:sc * 512], yo[:])

        # ---------------- fp16 patch (queries/keys 0..512) ----------------
        def patch_vproj():
            for st_i in range(SP // 128):
                ps = pqk.tile([128, 512], f32, tag="pq", name="pq")
                for d in range(DC):
                    nc.tensor.matmul(
                        ps[:], s_xt16[:, d, st_i * 128:(st_i + 1) * 128],
                        s_wv16[:, d, :], start=(d == 0), stop=(d == DC - 1))
                nc.vector.tensor_copy(
                    v16[:, st_i, :, 0:64],
                    ps.rearrange("p (h v) -> p h v", h=8))

        def patch_proj(hp, which):
            cols = slice(hp * 128, (hp + 1) * 128)
            for w_n, ctab, stab, dst in [((s_wq16, s_ck, s_sk, q16),
                                          (s_wk16, s_ck, s_sk, k16))[which]]:
                ps = pqk.tile([128, 512], f32, tag="pq", name="pq")
                for d in range(DC):
                    nc.tensor.matmul(
                        ps[:], w_n[:, d, cols], s_xt16[:, d, :],
                        start=(d == 0), stop=(d == DC - 1))
                sh = shf_pool.tile([128, 512], f32, tag="sh", name="sh")
                nc.vector.stream_shuffle(sh[:], ps[:], SWAP16)
                t = patch_tmp.tile([128, 512], f16, tag="pt", name="pt")
                m = patch_tmp.tile([128, 512], f16, tag="pm", name="pm")
                nc.vector.tensor_mul(t[:], ps[:], ctab[:, 0:SP])
                nc.vector.tensor_mul(m[:], sh[:], stab[:, 0:SP])
                nc.vector.tensor_add(dst[:, hp, :], t[:], m[:])

        patch_state = {}

        def patch_pv(hp, ps_o, pktp, pex, stop):
            for h in range(2):
                for half in range(2):
                    kt = 2 * pktp + half
                    nc.tensor.matmul(
                        ps_o[0:65, h * 512:(h + 1) * 512],
                        v16[:, kt, 2 * hp + h, 0:65],
                        pex[h][:, half, :],
                        start=(kt == 0), stop=(stop and half == 1))

        def patch_scores(hp, ktp):
            ps_s = [psc.tile([128, 2, 512], f32, tag="ps", name=f"ps{h}")
                    for h in range(2)]
            exs = [ex16_pool.tile([128, 2, 512], f16, tag="px",
                                  name=f"px{h}") for h in range(2)]
            for h in range(2):
                kq = slice(64 * h, 64 * (h + 1))
                for half in range(2):
                    kt = 2 * ktp + half
                    nc.tensor.matmul(
                        ps_s[h][:, half, :],
                        k16[kq, hp, kt * 128:(kt + 1) * 128],
                        q16[kq, hp, :], start=True, stop=False)
                    nc.tensor.matmul(
                        ps_s[h][:, half, :], bc2(s_id[:]),
                        bc2(s_m[:, kt, :]), perf_mode=DR,
                        start=False, stop=True)
            for h in range(2):
                if ktp == 1:
                    nc.gpsimd.memset(exs[h][:, :, 0:256], 0.0)
                    nc.scalar.activation(
                        out=exs[h][:, :, 256:512],
                        in_=ps_s[h][:, :, 256:512],
                        func=EXP, scale=0.125, bias=sbias[:])
                else:
                    nc.scalar.activation(
                        out=exs[h][:], in_=ps_s[h][:],
                        func=EXP, scale=0.125, bias=sbias[:])
            return exs

        def patch_piece(hp, step):
            # step 1: q proj+rope; 2: k proj+rope + scores ktp0;
            # step 3: scores ktp1 + both PVs + norm
            if step == 1:
                patch_proj(hp, 0)
            elif step == 2:
                patch_proj(hp, 1)
                patch_state[hp] = patch_scores(hp, 0)
            else:
                ps_o = po.tile([65, 1024], f32, tag="po", name="po")
                ex1 = patch_scores(hp, 1)
                patch_pv(hp, ps_o, 0, patch_state.pop(hp), False)
                patch_pv(hp, ps_o, 1, ex1, True)
                norm_chain(ps_o, hp, slice(0, SP), o16)

        def patch_e():
            for mt in range(DC):
                ps = pqk.tile([128, 512], f32, tag="pq", name="pq")
                for ec in range(E // 128):
                    nc.tensor.matmul(
                        ps[:], s_wo16[:, ec, mt * 128:(mt + 1) * 128],
                        o16[:, ec, :], start=(ec == 0), stop=(ec == E // 128 - 1))
                yo = yst_pool.tile([128, 512], f16, tag="yo", name="yo")
                nc.vector.tensor_copy(yo[:], ps[:])
                nc.sync.dma_start(yt2[:, mt, :], yo[:])

        # ---- emission schedule ----
        for st_i in range(8):
            vproj(st_i)
        for sc in range(4):
            projchunk(0, sc, do_q=(sc >= 2))

        for hp in range(NHP):
            for qc in range(1, 4):
                attention_qc(hp, qc)
                if hp == 0:
                    if qc == 1:
                        for st_i in range(8, 12):
                            vproj(st_i)
                        for sc in range(4, 6):
                            projchunk(0, sc, do_q=True)
                    elif qc == 2:
                        for st_i in range(12, 16):
                            vproj(st_i)
                        for sc in range(6, 8):
                            projchunk(0, sc, do_q=True)
                    else:
                        patch_vproj()
                if hp < NHP - 1:
                    # k chunks (8) + q chunks (6) for hp+1, spread over 3 qcs
                    lo, hi = [(0, 3), (3, 6), (6, 8)][qc - 1]
                    for sc in range(lo, hi):
                        projchunk(hp + 1, sc, do_q=(sc >= 2))
                    if qc == 3:
                        for sc in range(0, 2):
                            projchunk(hp + 1, sc, do_q=False)
                        # q for sc 0/1 not needed (qc=0 handled by patch)
                else:
                    phase_e(qc)
                # fp16 patch pieces ride inside later head-pairs' sections
                if hp == 1:
                    patch_piece(0, qc)
                elif hp == 2:
                    patch_piece(1, qc)
                    patch_piece(2, qc)
                elif hp == 3:
                    patch_piece(3, qc)
        patch_e()

    nc.compile()
    return nc


def _prepare_inputs(x, wq, wk, wv, wo, token_positions):
    """Build the 8 per-core input maps (all host-side layout shuffling)."""
    x = np.asarray(x, dtype=np.float32)
    wq = np.asarray(wq, dtype=np.float32)
    wk = np.asarray(wk, dtype=np.float32)
    wv = np.asarray(wv, dtype=np.float32)
    wo = np.asarray(wo, dtype=np.float32)
    pos = np.asarray(token_positions).astype(np.float32)

    # RoPE tables; rows 16-interleaved: [f0:16(x1), f0:16(x2), f16:32(x1),
    # f16:32(x2)] per 64-row head, tiled x2 over the 128 partitions.
    inv = THETA ** (-np.arange(0, DK, 2, dtype=np.float32) / DK)  # [32]
    ang = pos[:, None] * inv[None, :]                             # [S, 32]
    cosT = np.cos(ang).T.astype(np.float32)                       # [32, S]
    sinT = np.sin(ang).T.astype(np.float32)
    c16a, c16b = cosT[0:16], cosT[16:32]
    s16a, s16b = sinT[0:16], sinT[16:32]
    ctab = np.concatenate([c16a, c16a, c16b, c16b] * 2, 0)        # [128, S]
    stab = np.concatenate([-s16a, s16a, -s16b, s16b] * 2, 0)

    kk = np.arange(128)[:, None]
    qq = np.arange(512)[None, :]
    masks = np.stack(
        [np.where(qq < kk + 128 * i, MASK_NEG, 0.0) for i in range(4)], 0
    ).transpose(1, 0, 2).astype(F8)                               # [128, 4, 512]
    identity = (0.5 * np.eye(128)).astype(F8)

    def chunk_T(a):
        t = a.T  # [cols, rows]
        n = t.shape[0] // 128
        return np.ascontiguousarray(
            t.reshape(n, 128, t.shape[1]).transpose(1, 0, 2))

    # per head: x1a = dims 0,2,..,30; x2a = 1,3,..,31; x1b = 32,..,62;
    # x2b = 33,..,63 (16 rows each)
    def perm_rows(w, hf, swapped):
        out = np.empty((E, D), dtype=np.float32)
        for hp in range(NHP):
            for j, h in enumerate((hf * 8 + 2 * hp, hf * 8 + 2 * hp + 1)):
                r0 = 128 * hp + 64 * j
                blocks = [w[64 * h + 0:64 * h + 32:2],
                          w[64 * h + 1:64 * h + 32:2],
                          w[64 * h + 32:64 * h + 64:2],
                          w[64 * h + 33:64 * h + 64:2]]
                if swapped:
                    blocks = [blocks[1], blocks[0], blocks[3], blocks[2]]
                for g in range(4):
                    out[r0 + 16 * g:r0 + 16 * (g + 1)] = blocks[g]
        return out

    in_maps = []
    for core in range(N_CORES):
        b, hf = divmod(core, 2)
        woT = wo.T[E * hf:E * (hf + 1)]                    # [512, 1024]
        wot = np.ascontiguousarray(
            woT.reshape(4, 128, D).transpose(1, 0, 2))
        pq = chunk_T(perm_rows(wq, hf, False))
        pk = chunk_T(perm_rows(wk, hf, False))
        pv = chunk_T(wv[E * hf:E * (hf + 1)])
        xtc = chunk_T(x[b])
        in_maps.append({
            "xt": xtc.astype(F8),
            "wq": (0.5 * pq).astype(F8),
            "wqs": (0.5 * chunk_T(perm_rows(wq, hf, True))).astype(F8),
            "wk": pk.astype(F8),
            "wks": chunk_T(perm_rows(wk, hf, True)).astype(F8),
            "wv": pv.astype(F8),
            "wo": wot.astype(F8),
            "ctk": ctab.astype(np.float16),
            "stk": stab.astype(np.float16),
            "masks": masks, "ident": identity,
            "xt16": np.ascontiguousarray(xtc[:, :, 0:SP]).astype(np.float16),
            "wq16": pq.astype(np.float16),
            "wk16": pk.astype(np.float16),
            "wv16": pv.astype(np.float16),
            "wo16": wot.astype(np.float16),
        })
    return in_maps


def _assemble(results):
    out = np.zeros((B, S, D), dtype=np.float32)
    for core, res in enumerate(results):
        b = core // 2
        full = np.concatenate(
            [res["yt2"].astype(np.float32), res["yt"].astype(np.float32)],
            axis=2)                                        # [128, 8, 2048]
        part = full.transpose(1, 0, 2).reshape(D, S)       # [1024, 2048]
        out[b] += part.T
    return out


def get_program():
    if "nc" not in _cache:
        _cache["nc"] = _build_program()
    return _cache["nc"]


def kernel(x, wq, wk, wv, wo, token_positions):
    from concourse.bass_utils import run_bass_kernel_spmd

    nc = get_program()
    in_maps = _prepare_inputs(x, wq, wk, wv, wo, token_positions)
    res = run_bass_kernel_spmd(nc, in_maps, core_ids=list(range(N_CORES)))
    return _assemble(res.results)


# revision 13
# speedup vs baseline: 1.1507x; 1.1507x over previous
"""Fused causal MHA (RoPE) Trainium2 Bass kernel, SPMD over 8 NeuronCores.

Sharding: data-parallel over batch (4) x tensor-parallel over heads (2 groups
of 8 heads).  Core c handles batch c//2, heads (c%2)*8 .. +8.  Each core
computes a partial output (its 8 heads through the row-sharded Wo); the host
sums the two partials per batch.

v3 = fast fp8 DoubleRow pipeline for queries 512..2048 plus an fp16 "patch"
phase that recomputes queries 0..512 at high precision (attention there is
peaked on few keys, so fp8 quantization noise shows up directly in the
output; beyond 512 keys the softmax averaging washes it out).

fp8 main pipeline (all matmuls DoubleRow fp8e4m3, 0.5 cyc/elem):
  - projections contract d_model in pairs of 128-chunks (true DR pairs)
  - scores use the broadcast trick: both DR k-tiles alias the same k/q via
    stride-0 APs; q tables are pre-scaled by 0.5 so the x2 cancels
  - causal mask added with a 0.5*I stationary (broadcast DR), mask = -240
  - PV contracts two key tiles per instruction; row 64 of the stationary
    holds ones -> softmax denominators for free
  - output projection contracts E in DR pairs on fp8 o
RoPE: PE emits the projection and a row-swapped projection (row-permuted
fp8 weight copies); DVE needs only 2 muls + 1 add per chunk.  Head rows are
stored 16-interleaved ([x1a x2a x1b x2b] per head) so the patch phase can
swap partners with DVE stream_shuffle instead of a second projection.
exp on ACT (PSUM->SBUF, scale 1/8, bias -1.5); the q<256 quarter of each
diagonal 512-block skips exp (Pool memsets it).  Normalization: DVE copy +
reciprocal, Pool broadcast + multiply.  Phases interleave to keep PE fed.
"""

import numpy as np
import ml_dtypes

B = 4
S = 2048
D = 1024
H = 16
DK = 64
THETA = 10000.0
N_CORES = 8
E = 512          # per-core head-dim shard (8 heads * 64)
NHP = 4          # head pairs per core
ST = S // 128    # seq tiles of 128
DC = D // 128    # d_model chunks of 128
SP = 512         # patch length (queries 0..SP recomputed in fp16)
MASK_NEG = -240.0
F8 = ml_dtypes.float8_e4m3

_cache = {}


def _build_program():
    import concourse.tile as tile
    from concourse import bacc, mybir
    from contextlib import ExitStack

    f8 = mybir.dt.float8e4
    f16 = mybir.dt.float16
    f32 = mybir.dt.float32
    DR = mybir.MatmulPerfMode.DoubleRow
    EXP = mybir.ActivationFunctionType.Exp

    nc = bacc.Bacc("TRN2", target_bir_lowering=False, debug=False,
                   num_devices=N_CORES)

    def din(name, shape, dt=f8):
        return nc.dram_tensor(name, shape, dt, kind="ExternalInput").ap()

    xt = din("xt", [128, DC, S])               # x[b]^T as [128, dchunk, s]
    wq = din("wq", [128, DC, E])               # Wq' (permuted) ^T
    wqs = din("wqs", [128, DC, E])             # partner-swapped rows
    wk = din("wk", [128, DC, E])
    wks = din("wks", [128, DC, E])
    wv = din("wv", [128, DC, E])
    wo = din("wo", [128, E // 128, D])         # Wo'^T chunks [128e, 1024d]
    ctk = din("ctk", [128, S], f16)            # cos table
    stk = din("stk", [128, S], f16)            # signed sin table
    masks = din("masks", [128, 4, 512])        # additive causal masks (-240/0)
    ident = din("ident", [128, 128])           # 0.5*I
    # fp16 patch inputs
    xt16 = din("xt16", [128, DC, SP], f16)
    wq16 = din("wq16", [128, DC, E], f16)
    wk16 = din("wk16", [128, DC, E], f16)
    wv16 = din("wv16", [128, DC, E], f16)
    wo16 = din("wo16", [128, E // 128, D], f16)
    yt = nc.dram_tensor("yt", [128, DC, S - SP], f16, kind="ExternalOutput").ap()
    yt2 = nc.dram_tensor("yt2", [128, DC, SP], f16, kind="ExternalOutput").ap()

    with tile.TileContext(nc) as tc, ExitStack() as ctx:
        sb = ctx.enter_context(tc.tile_pool(name="sb", bufs=1))
        ex_pool = ctx.enter_context(tc.tile_pool(name="exps", bufs=6))
        ex16_pool = ctx.enter_context(tc.tile_pool(name="exps16", bufs=3))
        rope_tmp = ctx.enter_context(tc.tile_pool(name="rtmp", bufs=4))
        patch_tmp = ctx.enter_context(tc.tile_pool(name="ptmp", bufs=2))
        shf_pool = ctx.enter_context(tc.tile_pool(name="shf", bufs=2))
        oraw_pool = ctx.enter_context(tc.tile_pool(name="oraw", bufs=2))
        r_pool = ctx.enter_context(tc.tile_pool(name="rp", bufs=2))
        rb_pool = ctx.enter_context(tc.tile_pool(name="rbp", bufs=2))
        yst_pool = ctx.enter_context(tc.tile_pool(name="yst", bufs=3))

        # ---- resident SBUF tensors ----
        s_xt = sb.tile([128, DC, S], f8)
        s_wq = sb.tile([128, DC, E], f8)
        s_wqs = sb.tile([128, DC, E], f8)
        s_wk = sb.tile([128, DC, E], f8)
        s_wks = sb.tile([128, DC, E], f8)
        s_wv = sb.tile([128, DC, E], f8)
        s_wo = sb.tile([128, E // 128, D], f8)
        s_ck = sb.tile([128, S], f16)
        s_sk = sb.tile([128, S], f16)
        s_m = sb.tile([128, 4, 512], f8)
        s_id = sb.tile([128, 128], f8)
        v_aug = sb.tile([128, ST, 8, 80], f8)    # [k-part, ktile, head, dv+1]
        q_all = sb.tile([128, NHP, S], f8)
        k_all = sb.tile([128, NHP, S], f8)
        o_all = sb.tile([128, NHP, S], f8)       # normalized attn out
        sbias = sb.tile([128, 1], f32)
        # patch tensors
        s_xt16 = sb.tile([128, DC, SP], f16)
        s_wq16 = sb.tile([128, DC, E], f16)
        s_wk16 = sb.tile([128, DC, E], f16)
        s_wv16 = sb.tile([128, DC, E], f16)
        s_wo16 = sb.tile([128, E // 128, D], f16)
        v16 = sb.tile([128, SP // 128, 8, 72], f16)
        q16 = sb.tile([128, NHP, SP], f16)
        k16 = sb.tile([128, NHP, SP], f16)
        o16 = sb.tile([128, NHP, SP], f16)

        for d in range(DC):
            eng = nc.sync if d % 2 == 0 else nc.scalar
            eng.dma_start(s_xt[:, d, :], xt[:, d, :])
        nc.scalar.dma_start(s_wv[:], wv[:])
        for i, (dst, src) in enumerate(
                [(s_wq, wq), (s_wqs, wqs), (s_wk, wk), (s_wks, wks),
                 (s_ck, ctk), (s_sk, stk), (s_m, masks), (s_id, ident),
                 (s_xt16, xt16), (s_wq16, wq16), (s_wk16, wk16),
                 (s_wv16, wv16), (s_wo, wo), (s_wo16, wo16)]):
            eng = nc.sync if i % 2 == 0 else nc.scalar
            eng.dma_start(dst[:], src[:])

        nc.vector.memset(sbias[:], -1.5)
        nc.vector.memset(
            v_aug.rearrange("p a h c -> p (a h) c")[:, :, 64:65], 1.0)
        nc.vector.memset(
            v16.rearrange("p a h c -> p (a h) c")[:, :, 64:65], 1.0)

        pqk = ctx.enter_context(tc.tile_pool(name="pqk", bufs=2, space="PSUM"))
        psc = ctx.enter_context(tc.tile_pool(name="psc", bufs=2, space="PSUM"))
        po = ctx.enter_context(tc.tile_pool(name="po", bufs=1, space="PSUM"))

        SWAP16 = [i ^ 16 for i in range(32)]

        def bc2(ap):
            shp = list(ap.shape)
            for i in range(len(shp) - 1, 0, -1):
                if shp[i] == 1:
                    ap = ap.squeeze(i)
                    shp.pop(i)
            return ap.unsqueeze(1).broadcast_to([shp[0], 2] + shp[1:])

        def vproj(st_i):
            # v = x @ Wv^T for one 128-seq tile (natural layout), fp8 DR
            ps = pqk.tile([128, 512], f32, tag="pq", name="pq")
            for dp in range(DC // 2):
                nc.tensor.matmul(
                    ps[:], s_xt[:, 2 * dp:2 * dp + 2, st_i * 128:(st_i + 1) * 128],
                    s_wv[:, 2 * dp:2 * dp + 2, :], perf_mode=DR,
                    start=(dp == 0), stop=(dp == DC // 2 - 1))
            nc.vector.tensor_copy(
                v_aug[:, st_i, :, 0:64],
                ps.rearrange("p (h v) -> p h v", h=8))

        def projchunk(hp, sc, do_q):
            # fp8 q and/or k (+ row-swapped copies) for one 256-chunk + RoPE
            sl = slice(sc * 256, (sc + 1) * 256)
            cols = slice(hp * 128, (hp + 1) * 128)
            plan = ([(s_wq, s_wqs, q_all)] if do_q else []) + \
                   [(s_wk, s_wks, k_all)]
            for w_n, w_s, dst in plan:
                ps = pqk.tile([128, 512], f32, tag="pq", name="pq")
                for dp in range(DC // 2):
                    nc.tensor.matmul(
                        ps[:, 0:256], w_n[:, 2 * dp:2 * dp + 2, cols],
                        s_xt[:, 2 * dp:2 * dp + 2, sl], perf_mode=DR,
                        start=(dp == 0), stop=(dp == DC // 2 - 1))
                for dp in range(DC // 2):
                    nc.tensor.matmul(
                        ps[:, 256:512], w_s[:, 2 * dp:2 * dp + 2, cols],
                        s_xt[:, 2 * dp:2 * dp + 2, sl], perf_mode=DR,
                        start=(dp == 0), stop=(dp == DC // 2 - 1))
                t = rope_tmp.tile([128, 256], f16, tag="rt", name="rt")
                m = rope_tmp.tile([128, 256], f16, tag="rm", name="rm")
                nc.vector.tensor_mul(t[:], ps[:, 0:256], s_ck[:, sl])
                nc.vector.tensor_mul(m[:], ps[:, 256:512], s_sk[:, sl])
                nc.vector.tensor_add(dst[:, hp, sl], t[:], m[:])

        def norm_chain(ps_o, hp, qc_sl, dst):
            o_raw = oraw_pool.tile([65, 1024], f32, tag="or", name="or")
            nc.vector.tensor_copy(o_raw[:], ps_o[:])       # frees the bank
            r = r_pool.tile([1, 1024], f32, tag="r", name="r")
            nc.vector.reciprocal(r[:], o_raw[64:65, :])
            rb = rb_pool.tile([64, 1024], f32, tag="rb", name="rb")
            nc.gpsimd.partition_broadcast(rb[:], r[:])
            for h in range(2):
                nc.gpsimd.tensor_mul(
                    dst[64 * h:64 * (h + 1), hp, qc_sl],
                    o_raw[0:64, h * 512:(h + 1) * 512],
                    rb[:, h * 512:(h + 1) * 512])

        def attention_qc(hp, qc):
            # fp8 attention for queries qc*512..(qc+1)*512, qc in 1..3
            ps_o = po.tile([65, 1024], f32, tag="po", name="po")
            prev = None

            def pv(pktp, pex, stop):
                for h in range(2):
                    nc.tensor.matmul(
                        ps_o[0:65, h * 512:(h + 1) * 512],
                        v_aug[:, 2 * pktp:2 * pktp + 2, 2 * hp + h, 0:65],
                        pex[h][:], perf_mode=DR,
                        start=(pktp == 0), stop=stop)

            for ktp in range(2 * (qc + 1)):
                ps_s = [psc.tile([128, 2, 512], f32, tag="ps", name=f"ps{h}")
                        for h in range(2)]
                exs = [ex_pool.tile([128, 2, 512], f8, tag="ex", name=f"ex{h}")
                       for h in range(2)]
                for h in range(2):
                    kq = slice(64 * h, 64 * (h + 1))
                    for half in range(2):
                        kt = 2 * ktp + half
                        d = kt - 4 * qc
                        nc.tensor.matmul(
                            ps_s[h][:, half, :],
                            bc2(k_all[kq, hp, kt * 128:(kt + 1) * 128]),
                            bc2(q_all[kq, hp, qc * 512:(qc + 1) * 512]),
                            perf_mode=DR, start=True, stop=(d < 0))
                        if d >= 0:
                            nc.tensor.matmul(
                                ps_s[h][:, half, :], bc2(s_id[:]),
                                bc2(s_m[:, d, :]), perf_mode=DR,
                                start=False, stop=True)
                for h in range(2):
                    if ktp == 2 * qc + 1:
                        nc.gpsimd.memset(exs[h][:, :, 0:256], 0.0)
                        nc.scalar.activation(
                            out=exs[h][:, :, 256:512],
                            in_=ps_s[h][:, :, 256:512],
                            func=EXP, scale=0.125, bias=sbias[:])
                    else:
                        nc.scalar.activation(
                            out=exs[h][:], in_=ps_s[h][:],
                            func=EXP, scale=0.125, bias=sbias[:])
                if prev is not None:
                    pv(prev[0], prev[1], False)
                prev = (ktp, exs)
            pv(prev[0], prev[1], True)
            norm_chain(ps_o, hp, slice(qc * 512, (qc + 1) * 512), o_all)

        def phase_e(sc):
            # fp8 DR output projection for seq chunk sc (1..3) -> yt
            sl = slice(sc * 512, (sc + 1) * 512)
            for mt in range(DC):
                ps = pqk.tile([128, 512], f32, tag="pq", name="pq")
                for ep in range(2):
                    nc.tensor.matmul(
                        ps[:], s_wo[:, 2 * ep:2 * ep + 2, mt * 128:(mt + 1) * 128],
                        o_all[:, 2 * ep:2 * ep + 2, sl], perf_mode=DR,
                        start=(ep == 0), stop=(ep == 1))
                yo = yst_pool.tile([128, 512], f16, tag="yo", name="yo")
                nc.vector.tensor_copy(yo[:], ps[:])
                nc.sync.dma_start(yt[:, mt, (sc - 1) * 512:sc * 512], yo[:])

        # ---------------- fp16 patch (queries/keys 0..512) ----------------
        def patch_vproj():
            for st_i in range(SP // 128):
                ps = pqk.tile([128, 512], f32, tag="pq", name="pq")
                for d in range(DC):
                    nc.tensor.matmul(
                        ps[:], s_xt16[:, d, st_i * 128:(st_i + 1) * 128],
                        s_wv16[:, d, :], start=(d == 0), stop=(d == DC - 1))
                nc.vector.tensor_copy(
                    v16[:, st_i, :, 0:64],
                    ps.rearrange("p (h v) -> p h v", h=8))

        def patch_proj(hp, which):
            cols = slice(hp * 128, (hp + 1) * 128)
            for w_n, ctab, stab, dst in [((s_wq16, s_ck, s_sk, q16),
                                          (s_wk16, s_ck, s_sk, k16))[which]]:
                ps = pqk.tile([128, 512], f32, tag="pq", name="pq")
                for d in range(DC):
                    nc.tensor.matmul(
                        ps[:], w_n[:, d, cols], s_xt16[:, d, :],
                        start=(d == 0), stop=(d == DC - 1))
                sh = shf_pool.tile([128, 512], f32, tag="sh", name="sh")
                nc.vector.stream_shuffle(sh[:], ps[:], SWAP16)
                t = patch_tmp.tile([128, 512], f16, tag="pt", name="pt")
                m = patch_tmp.tile([128, 512], f16, tag="pm", name="pm")
                nc.vector.tensor_mul(t[:], ps[:], ctab[:, 0:SP])
                nc.vector.tensor_mul(m[:], sh[:], stab[:, 0:SP])
                nc.vector.tensor_add(dst[:, hp, :], t[:], m[:])

        patch_state = {}

        def patch_pv(hp, ps_o, pktp, pex, stop):
            for h in range(2):
                for half in range(2):
                    kt = 2 * pktp + half
                    nc.tensor.matmul(
                        ps_o[0:65, h * 512:(h + 1) * 512],
                        v16[:, kt, 2 * hp + h, 0:65],
                        pex[h][:, half, :],
                        start=(kt == 0), stop=(stop and half == 1))

        def patch_scores(hp, ktp):
            ps_s = [psc.tile([128, 2, 512], f32, tag="ps", name=f"ps{h}")
                    for h in range(2)]
            exs = [ex16_pool.tile([128, 2, 512], f16, tag="px",
                                  name=f"px{h}") for h in range(2)]
            for h in range(2):
                kq = slice(64 * h, 64 * (h + 1))
                for half in range(2):
                    kt = 2 * ktp + half
                    nc.tensor.matmul(
                        ps_s[h][:, half, :],
                        k16[kq, hp, kt * 128:(kt + 1) * 128],
                        q16[kq, hp, :], start=True, stop=False)
                    nc.tensor.matmul(
                        ps_s[h][:, half, :], bc2(s_id[:]),
                        bc2(s_m[:, kt, :]), perf_mode=DR,
                        start=False, stop=True)
            for h in range(2):
                if ktp == 1:
                    nc.gpsimd.memset(exs[h][:, :, 0:256], 0.0)
                    nc.scalar.activation(
                        out=exs[h][:, :, 256:512],
                        in_=ps_s[h][:, :, 256:512],
                        func=EXP, scale=0.125, bias=sbias[:])
                else:
                    nc.scalar.activation(
                        out=exs[h][:], in_=ps_s[h][:],
                        func=EXP, scale=0.125, bias=sbias[:])
            return exs

        def patch_piece(hp, step):
            # step 1: q proj+rope; 2: k proj+rope + scores ktp0;
            # step 3: scores ktp1 + both PVs + norm
            if step == 1:
                patch_proj(hp, 0)
            elif step == 2:
                patch_proj(hp, 1)
                patch_state[hp] = patch_scores(hp, 0)
            else:
                ps_o = po.tile([65, 1024], f32, tag="po", name="po")
                ex1 = patch_scores(hp, 1)
                patch_pv(hp, ps_o, 0, patch_state.pop(hp), False)
                patch_pv(hp, ps_o, 1, ex1, True)
                norm_chain(ps_o, hp, slice(0, SP), o16)

        def patch_e():
            for mt in range(DC):
                ps = pqk.tile([128, 512], f32, tag="pq", name="pq")
                for ec in range(E // 128):
                    nc.tensor.matmul(
                        ps[:], s_wo16[:, ec, mt * 128:(mt + 1) * 128],
                        o16[:, ec, :], start=(ec == 0), stop=(ec == E // 128 - 1))
                yo = yst_pool.tile([128, 512], f16, tag="yo", name="yo")
                nc.vector.tensor_copy(yo[:], ps[:])
                nc.sync.dma_start(yt2[:, mt, :], yo[:])

        # ---- emission schedule ----
        for st_i in range(8):
            vproj(st_i)
        for sc in range(4):
            projchunk(0, sc, do_q=(sc >= 2))

        for hp in range(NHP):
            for qc in range(1, 4):
                attention_qc(hp, qc)
                if hp == 0:
                    if qc == 1:
                        for st_i in range(8, 12):
                            vproj(st_i)
                        for sc in range(4, 6):
                            projchunk(0, sc, do_q=True)
                    elif qc == 2:
                        for st_i in range(12, 16):
                            vproj(st_i)
                        for sc in range(6, 8):
                            projchunk(0, sc, do_q=True)
                    else:
                        patch_vproj()
                if hp < NHP - 1:
                    # k chunks (8) + q chunks (6) for hp+1, spread over 3 qcs
                    lo, hi = [(0, 3), (3, 6), (6, 8)][qc - 1]
                    for sc in range(lo, hi):
                        projchunk(hp + 1, sc, do_q=(sc >= 2))
                    if qc == 3:
                        for sc in range(0, 2):
                            projchunk(hp + 1, sc, do_q=False)
                        # q for sc 0/1 not needed (qc=0 handled by patch)
                else:
                    phase_e(qc)
            # fp16 patch blocks at head-pair boundaries (po pool is single-
            # buffered: interleaving norm chains inside a section stalls PE)
            if hp == 0:
                for s in (1, 2, 3):
                    patch_piece(0, s)
            elif hp == 1:
                for s in (1, 2, 3):
                    patch_piece(1, s)
            elif hp == 2:
                for s in (1, 2, 3):
                    patch_piece(2, s)
                for s in (1, 2, 3):
                    patch_piece(3, s)
        patch_e()

    nc.compile()
    return nc


def _prepare_inputs(x, wq, wk, wv, wo, token_positions):
    """Build the 8 per-core input maps (all host-side layout shuffling)."""
    x = np.asarray(x, dtype=np.float32)
    wq = np.asarray(wq, dtype=np.float32)
    wk = np.asarray(wk, dtype=np.float32)
    wv = np.asarray(wv, dtype=np.float32)
    wo = np.asarray(wo, dtype=np.float32)
    pos = np.asarray(token_positions).astype(np.float32)

    # RoPE tables; rows 16-interleaved: [f0:16(x1), f0:16(x2), f16:32(x1),
    # f16:32(x2)] per 64-row head, tiled x2 over the 128 partitions.
    inv = THETA ** (-np.arange(0, DK, 2, dtype=np.float32) / DK)  # [32]
    ang = pos[:, None] * inv[None, :]                             # [S, 32]
    cosT = np.cos(ang).T.astype(np.float32)                       # [32, S]
    sinT = np.sin(ang).T.astype(np.float32)
    c16a, c16b = cosT[0:16], cosT[16:32]
    s16a, s16b = sinT[0:16], sinT[16:32]
    ctab = np.concatenate([c16a, c16a, c16b, c16b] * 2, 0)        # [128, S]
    stab = np.concatenate([-s16a, s16a, -s16b, s16b] * 2, 0)

    kk = np.arange(128)[:, None]
    qq = np.arange(512)[None, :]
    masks = np.stack(
        [np.where(qq < kk + 128 * i, MASK_NEG, 0.0) for i in range(4)], 0
    ).transpose(1, 0, 2).astype(F8)                               # [128, 4, 512]
    identity = (0.5 * np.eye(128)).astype(F8)

    def chunk_T(a):
        t = a.T  # [cols, rows]
        n = t.shape[0] // 128
        return np.ascontiguousarray(
            t.reshape(n, 128, t.shape[1]).transpose(1, 0, 2))

    # per head: x1a = dims 0,2,..,30; x2a = 1,3,..,31; x1b = 32,..,62;
    # x2b = 33,..,63 (16 rows each)
    def perm_rows(w, hf, swapped):
        out = np.empty((E, D), dtype=np.float32)
        for hp in range(NHP):
            for j, h in enumerate((hf * 8 + 2 * hp, hf * 8 + 2 * hp + 1)):
                r0 = 128 * hp + 64 * j
                blocks = [w[64 * h + 0:64 * h + 32:2],
                          w[64 * h + 1:64 * h + 32:2],
                          w[64 * h + 32:64 * h + 64:2],
                          w[64 * h + 33:64 * h + 64:2]]
                if swapped:
                    blocks = [blocks[1], blocks[0], blocks[3], blocks[2]]
                for g in range(4):
                    out[r0 + 16 * g:r0 + 16 * (g + 1)] = blocks[g]
        return out

    in_maps = []
    for core in range(N_CORES):
        b, hf = divmod(core, 2)
        woT = wo.T[E * hf:E * (hf + 1)]                    # [512, 1024]
        wot = np.ascontiguousarray(
            woT.reshape(4, 128, D).transpose(1, 0, 2))
        pq = chunk_T(perm_rows(wq, hf, False))
        pk = chunk_T(perm_rows(wk, hf, False))
        pv = chunk_T(wv[E * hf:E * (hf + 1)])
        xtc = chunk_T(x[b])
        in_maps.append({
            "xt": xtc.astype(F8),
            "wq": (0.5 * pq).astype(F8),
            "wqs": (0.5 * chunk_T(perm_rows(wq, hf, True))).astype(F8),
            "wk": pk.astype(F8),
            "wks": chunk_T(perm_rows(wk, hf, True)).astype(F8),
            "wv": pv.astype(F8),
            "wo": wot.astype(F8),
            "ctk": ctab.astype(np.float16),
            "stk": stab.astype(np.float16),
            "masks": masks, "ident": identity,
            "xt16": np.ascontiguousarray(xtc[:, :, 0:SP]).astype(np.float16),
            "wq16": pq.astype(np.float16),
            "wk16": pk.astype(np.float16),
            "wv16": pv.astype(np.float16),
            "wo16": wot.astype(np.float16),
        })
    return in_maps


def _assemble(results):
    out = np.zeros((B, S, D), dtype=np.float32)
    for core, res in enumerate(results):
        b = core // 2
        full = np.concatenate(
            [res["yt2"].astype(np.float32), res["yt"].astype(np.float32)],
            axis=2)                                        # [128, 8, 2048]
        part = full.transpose(1, 0, 2).reshape(D, S)       # [1024, 2048]
        out[b] += part.T
    return out


def get_program():
    if "nc" not in _cache:
        _cache["nc"] = _build_program()
    return _cache["nc"]


def kernel(x, wq, wk, wv, wo, token_positions):
    from concourse.bass_utils import run_bass_kernel_spmd

    nc = get_program()
    in_maps = _prepare_inputs(x, wq, wk, wv, wo, token_positions)
    res = run_bass_kernel_spmd(nc, in_maps, core_ids=list(range(N_CORES)))
    return _assemble(res.results)
